# revision 64
# baseline (speedup 1.0000x reference)
# Trainium2 Bass kernel for nn_MultiHeadAttention (B=2, S=2048, D=1024, H=16).
#
# Sharding: batch+head tensor-parallel over 8 cores. Core c handles batch
# c//4 and head-group c%4 (4 heads, 256 e-dims): column-sharded wq/wk/wv,
# row-sharded wo with the partial-output sum done on the host. Each core
# only reads its batch's q/k/v (12MB fp16 -> 6MB fp8 per core) and writes
# a [2048, 1024] fp16 partial.
#
# Projections run as fp8e4 DoubleRow matmuls (2 contraction tiles of 128
# per pass, 0.5 cyc/row): q/k/v stream fp8 from the host, wq/wk (and bq)
# pre-scaled by 16 so the fp8 weights stay clear of subnormals; the 16*16
# score scaling is folded into the softmax exp scale (0.125/256).
#
# Attention stays fp16 (DoubleRow dst must start at partition 0, which
# makes the Z ones-row trick impossible in DR mode):
#   QT/KT = (e 128, 2 e-tiles, 2048) computed via DR with weight tiles
#   stationary; V in natural (token, e) layout packed [V_h | ones] per
#   (kt, head) so AV yields row sums (Z) free in psum row 64.
#   ScoresT = (k, q) per head; exp PSUM->SBUF on ACT gives P^T directly.
#   Causal-boundary blocks multiply P^T with a resident 0/1 triangle tile
#   on DVE (fp16 2x mode).
# Bias handling: K bias dropped (softmax per-query shift invariance),
#   V bias folded into the host-side output bias (out += wo @ bv),
#   Q bias added on DVE during the PSUM->SBUF copy.
# Host sums the 4 partial outputs per batch in fp32 and adds bo + wo@bv.

import numpy as np

B, S, D, H = 2, 2048, 1024, 16
DK = D // H            # 64
NC = 8                 # cores
NH = 4                 # heads per core
EL = NH * DK           # 256 local e-dims
NCH = 4                # projection token-chunks per core
CH = S // NCH          # 512
NDT = D // 128         # 8 contraction tiles
NDP = NDT // 2         # 4 DoubleRow contraction pairs
NKT = S // 128         # 16 k-tiles
NQB = S // 512         # 4 q-blocks

SKIP, PLAIN = -1, -2   # block classes (>=0 means partial-pattern index)


def _classify_mask(mask):
    """Per (kt, qj) block classification of the (S_q, S_k) mask.

    Returns cls[kt][qj] (SKIP / PLAIN / pattern idx), rng[kt][qj] live col
    range, pr[kt][qj] partial col range, and the deduped 0/1 patterns
    (list of [128, w] float16 arrays) for the partial ranges."""
    m = np.asarray(mask).reshape(S, S)              # [q, k]; 0 = masked
    liveT = (m != 0).T                              # [k, q]
    cls = [[PLAIN] * NQB for _ in range(NKT)]
    rng = [[(0, 512)] * NQB for _ in range(NKT)]
    pr = [[(0, 0)] * NQB for _ in range(NKT)]
    uniq = {}
    pats = []
    for kt in range(NKT):
        for qj in range(NQB):
            blk = liveT[kt * 128:(kt + 1) * 128, qj * 512:(qj + 1) * 512]
            if blk.all():
                cls[kt][qj] = PLAIN
            elif not blk.any():
                cls[kt][qj] = SKIP
            else:
                live_col = blk.any(axis=0)
                nz = np.nonzero(live_col)[0]
                c0, c1 = int(nz[0]), int(nz[-1]) + 1
                rng[kt][qj] = (c0, c1)
                part_col = live_col & ~blk.all(axis=0)
                pz = np.nonzero(part_col)[0]
                p0, p1 = int(pz[0]), int(pz[-1]) + 1
                pr[kt][qj] = (p0, p1)
                pat = blk[:, p0:p1].astype(np.float16)
                key = (p1 - p0, pat.tobytes())
                if key not in uniq:
                    uniq[key] = len(pats)
                    pats.append(np.ascontiguousarray(pat))
                cls[kt][qj] = uniq[key]
    return cls, rng, pr, pats


def _build_program(cls, rng, pr, pat_widths):
    import concourse.bacc as bacc
    import concourse.mybir as mybir
    from concourse.tile import TileContext

    f32 = mybir.dt.float32
    f16 = mybir.dt.float16
    f8 = mybir.dt.float8e4
    DR = mybir.MatmulPerfMode.DoubleRow
    Exp = mybir.ActivationFunctionType.Exp
    mult = mybir.AluOpType.mult
    ESC = 0.125            # exp scale: 1/sqrt(DK)

    # pattern offsets inside the resident mask tile
    moff = []
    o = 0
    for w in pat_widths:
        moff.append(o)
        o += w
    MW = max(o, 1)

    nc = bacc.Bacc("TRN2", target_bir_lowering=False, debug=False,
                   num_devices=NC)

    # q/k/v packed in one tensor so a projection chunk is a single DMA
    # (the per-DMA issue pipeline -- seq config + HWDGE + completion
    # semaphore -- costs ~1.5us, which dominated the startup)
    qkv = nc.dram_tensor("qkv", [3 * D, S], f16, kind="ExternalInput")
    w3d = nc.dram_tensor("w3", [128, 3 * NDT * EL], f16,
                         kind="ExternalInput")
    bqd = nc.dram_tensor("bq", [128, 2], f32, kind="ExternalInput")
    auxd = nc.dram_tensor("aux", [128, MW + 2 * D], f16,
                          kind="ExternalInput")
    out = nc.dram_tensor("out", [S, D], f16, kind="ExternalOutput")

    # transposed-input view: [p, j, t, c] with t the 128-row d-block
    qkv_r = qkv.ap().rearrange("(j t p) c -> p j t c", j=3, p=128)
    w3_r = w3d.ap().rearrange("p (j t e) -> p j t e", j=3, t=NDT)

    with TileContext(nc) as tc:
        with (
            tc.tile_pool(name="const", bufs=1) as constp,
            tc.tile_pool(name="per", bufs=1) as perp,
            tc.tile_pool(name="stage", bufs=2) as stagep,
            tc.tile_pool(name="pt", bufs=18) as ptp,
            tc.tile_pool(name="zz", bufs=4) as zzp,
            tc.tile_pool(name="zb", bufs=6) as zbp,
            tc.tile_pool(name="ost", bufs=4) as ostp,
            tc.tile_pool(name="psA", bufs=2, space="PSUM") as psA,
            tc.tile_pool(name="psS", bufs=2, space="PSUM") as psS,
            tc.tile_pool(name="psO", bufs=2, space="PSUM") as psO,
        ):
            # ---- constants ----
            w3 = constp.tile([128, 3, NDT, EL], f16, tag="w3")
            bq_sb = constp.tile([128, 2], f32, tag="bq")
            aux_sb = constp.tile([128, MW + 2 * D], f16, tag="aux")
            msk = aux_sb[:, 0:MW]
            woT_sb = aux_sb[:, MW:].rearrange("p (i e) -> p i e", i=2)

            # ---- persistent activations ----
            # QT/KT/OT: [p, e-tile, token]; head h -> (rows (h%2)*64,
            # e-tile h//2)
            QT_sb = perp.tile([128, 2, S], f16, tag="QT")
            KT_sb = perp.tile([128, 2, S], f16, tag="KT")
            OT_sb = perp.tile([128, 2, S], f16, tag="OT")
            # V natural (token, e) packed per (kt, head) as
            # [ones-col | 63 zero cols | V_h(64)] (128 cols): Z (the exp
            # row sum) lands in psum row 0 where the DVE reciprocal can
            # read it without a staging copy, and the AV rows sit at
            # partitions 64:128 (64-partition engine APs must start at 0
            # or 64 on hardware).
            V_big = perp.tile([128, NKT * NH * 128], f16, tag="Vb")
            V3 = V_big[:].rearrange("p (t x) -> p t x", x=128)

            def emit_vz():
                nc.vector.memset(V3[:, :, 0:64], 0.0)
                nc.vector.memset(V3[:, :, 0:1], 1.0)

            # ---- projections (fp8 DoubleRow, 2 d-tiles per pass) ----
            # generators yield after each PE quantum (~0.2-0.4us) so the
            # attention loop can drain them as PE filler between groups
            JIDX = {"q": 0, "k": 1, "v": 2}

            def stage_chunk(c):
                # whole q/k/v chunk in ONE DMA from the packed qkv tensor
                lo = c * CH
                st3 = stagep.tile([128, 3, NDT, CH], f16, tag="stage")
                nc.sync.dma_start(out=st3[:], in_=qkv_r[:, :, :, lo:lo + CH])
                return st3

            def gen_qk_ep(name, c, ep, st, act=False):
                # transposed layout [e, token]: out [128 e, 512] per
                # e-tile, weight tiles stationary. act=True routes the
                # PSUM->SBUF copy to the ACT engine (used in the startup
                # where DVE is the bottleneck and ACT is idle).
                w = w3[:, JIDX[name]]
                lo = c * CH
                ps = psA.tile([128, 512], f32, tag="proj",
                              name=f"{name}p{c}{ep}")
                for t in range(NDT):
                    nc.tensor.matmul(
                        ps[:], w[:, t, ep * 128:(ep + 1) * 128],
                        st[:, t, :],
                        start=(t == 0), stop=(t == NDT - 1))
                    if t == 3:
                        yield
                if name == "q":
                    if act:
                        nc.scalar.add(QT_sb[:, ep, lo:lo + CH], ps[:],
                                      bq_sb[:, ep:ep + 1])
                    else:
                        nc.vector.tensor_scalar_add(
                            QT_sb[:, ep, lo:lo + CH], ps[:],
                            bq_sb[:, ep:ep + 1])
                elif act:
                    nc.scalar.copy(KT_sb[:, ep, lo:lo + CH], ps[:])
                else:
                    nc.vector.tensor_copy(KT_sb[:, ep, lo:lo + CH], ps[:])
                yield

            def gen_v(c, st, act=False):
                # natural layout: out [64 tok, 256 e] per subtile; a psum
                # [128, 512] holds one kt (128 tokens) per col-half
                w = w3[:, 2]
                for half in range(2):          # 2 kt per psum tile
                    ps = psA.tile([128, 512], f32, tag="proj",
                                  name=f"vp{c}{half}")
                    for kk in range(2):        # kt within this psum tile
                        tt = half * 2 + kk
                        sub = ps[:, kk * 256:(kk + 1) * 256]
                        for t in range(NDT):
                            nc.tensor.matmul(
                                sub, st[:, t, tt * 128:(tt + 1) * 128],
                                w[:, t, :],
                                start=(t == 0), stop=(t == NDT - 1))
                        yield
                    for kk in range(2):
                        kt = c * 4 + half * 2 + kk
                        dst = V3[:, kt * NH:(kt + 1) * NH, 64:128]
                        src = ps[:, kk * 256:(kk + 1) * 256].rearrange(
                            "p (h e) -> p h e", h=NH)
                        if act:
                            nc.scalar.copy(dst, src)
                        else:
                            nc.vector.tensor_copy(dst, src)

            def gen_chunk3(c, staged=None, act=False):
                # e-tile 0 of q AND k first: the next q-block's pair-A
                # scores need exactly those
                st3 = stage_chunk(c) if staged is None else staged
                yield from gen_qk_ep("q", c, 0, st3[:, 0], act=act)
                yield from gen_qk_ep("k", c, 0, st3[:, 1], act=act)
                yield from gen_qk_ep("q", c, 1, st3[:, 0], act=act)
                yield from gen_qk_ep("k", c, 1, st3[:, 1], act=act)
                yield from gen_v(c, st3[:, 2], act=act)

            def chain(*gens):
                for g in gens:
                    yield from g

            def flag_end(gen, flag):
                yield from gen
                flag[0] = True

            def drain(g):
                for _ in g:
                    pass

            # ---- attention ----
            # Per q-block, heads processed as two interleaved pairs so PE
            # always has the other head's matmuls while ACT runs exp. The
            # AV matmul lags scores by one group so it never waits on exp,
            # and a caller-supplied filler (projection / oproj pieces) is
            # drained between groups to keep the PE queue dense.
            def emit_attention(qj, pairs=(0, 1), defer_oproj=False,
                               tail=False, kw=False, filler=None, pulls=1,
                               act_oc=False, defer_av=False, av_gate=None):
                qlo = qj * 512
                acts = [kt for kt in range(NKT) if cls[kt][qj] != SKIP]
                if not acts:
                    return
                groups = [acts[gi:gi + 2] for gi in range(0, len(acts), 2)]

                def pull_filler(n):
                    for _ in range(n):
                        if filler is not None:
                            try:
                                next(filler)
                                continue
                            except StopIteration:
                                pass
                        if kw:
                            emit_keepwarm(2, 700 + qj * 100
                                          + 3 * pull_filler.kwn)
                            pull_filler.kwn += 1
                        break
                pull_filler.kwn = 0

                def emit_scores(grp, h, sc):
                    ep, hp = h // 2, (h % 2) * 64
                    for i, kt in enumerate(grp):
                        c0, c1 = rng[kt][qj]
                        klo = kt * 128
                        nc.tensor.matmul(
                            sc[:, i * 512 + c0:i * 512 + c1],
                            KT_sb[hp:hp + 64, ep, klo:klo + 128],
                            QT_sb[hp:hp + 64, ep, qlo + c0:qlo + c1],
                            start=True, stop=True)

                def emit_exp_mask(grp, pt, sc):
                    spans = [(i * 512 + rng[kt][qj][0],
                              i * 512 + rng[kt][qj][1])
                             for i, kt in enumerate(grp)]
                    lo, hi2 = spans[0][0], spans[-1][1]
                    dead = (hi2 - lo) - sum(b - a for a, b in spans)
                    exp_spans = spans if dead > 0 else [(lo, hi2)]
                    for a, bnd in exp_spans:
                        nc.scalar.activation(pt[:, a:bnd], sc[:, a:bnd],
                                             Exp, scale=ESC)
                    for i, kt in enumerate(grp):
                        cl = cls[kt][qj]
                        if cl >= 0:
                            pp0, pp1 = pr[kt][qj]
                            sl = slice(i * 512 + pp0, i * 512 + pp1)
                            nc.vector.tensor_tensor(
                                pt[:, sl], pt[:, sl],
                                msk[:, moff[cl]:moff[cl] + pp1 - pp0],
                                op=mult)

                def emit_av(grp, hi, h, pt, ots, n_done):
                    for i, kt in enumerate(grp):
                        c0, c1 = rng[kt][qj]
                        base = (kt * NH + h) * 128
                        vap = V_big[:, base:base + 128]
                        n_done[hi] += 1
                        nc.tensor.matmul(
                            ots[hi][:, c0:c1], vap,
                            pt[:, i * 512 + c0:i * 512 + c1],
                            start=(n_done[hi] == 1),
                            stop=(n_done[hi] == len(acts)))

                def emit_norm(pair, hi, h, ots):
                    # normalize: psum row 0 of ot = Z (the ones column),
                    # rows 64:128 = AV. The reciprocal reads Z straight
                    # from PSUM partition 0 (no staging copy); the AV rows
                    # move to SBUF base 0 so the normalize tensor_tensor
                    # sees equal base partitions on its two SBUF inputs.
                    ep, hp = h // 2, (h % 2) * 64
                    ot = ots[hi]
                    on_act = act_oc or (tail and pair == pairs[-1])
                    z = zzp.tile([1, 512], f32, tag="z")
                    if on_act:
                        nc.scalar.copy(z[:], ot[0:1, :])
                    else:
                        nc.vector.tensor_copy(z[:], ot[0:1, :])
                    rz = zzp.tile([1, 512], f32, tag="z")
                    nc.vector.reciprocal_approx_fast(rz[:], z[:])
                    oc = zbp.tile([64, 512], f32, tag="oc")
                    if on_act:
                        # ACT is idle in the startup and the tail
                        nc.scalar.copy(oc[:], ot[64:128, :])
                    else:
                        nc.vector.tensor_copy(oc[:], ot[64:128, :])
                    rb = zbp.tile([64, 512], f32, tag="zb")
                    nc.gpsimd.partition_broadcast(rb[:], rz[:],
                                                  channels=64)
                    if tail and pair == pairs[-1]:
                        # keep the PE clock warm through the norm chain:
                        # these matmuls depend on oc so they execute
                        # spread across the DVE/Pool chain, not up front
                        for kwi in range(2):
                            kwt = psA.tile([64, 512], f32, tag="proj",
                                           name=f"kwn{qj}{h}{kwi}")
                            nc.tensor.matmul(
                                kwt[:, 0:64], oc[:, 0:64],
                                oc[:, 0:64], start=True, stop=True)
                    nc.vector.tensor_tensor(
                        OT_sb[hp:hp + 64, ep, qlo:qlo + 512],
                        oc[:, :], rb[:], op=mult)

                # Flat item stream over both pairs. The AV for an item
                # lags its exp so the PE never waits on ACT; across the
                # pair boundary the lag stretches to 2 so pair A's last
                # AVs + norms are all emitted BEFORE pair B's first AV
                # touches the psO ring slot that pair A's norms still
                # read (emission order is dependency order).
                stream = [(pair, grp, hi) for pair in pairs
                          for grp in groups for hi in range(2)]

                ots = {}
                n_done = {}
                n_av = {p: 0 for p in pairs}
                pend = []   # (pair, grp, hi, h, pt)

                def flush_av():
                    pair, grp, hi, h, pt = pend.pop(0)
                    emit_av(grp, hi, h, pt, ots[pair], n_done[pair])
                    n_av[pair] += len(grp)
                    if n_av[pair] == 2 * len(acts):
                        for hj in range(2):
                            emit_norm(pair, hj, pair * 2 + hj, ots[pair])

                for idx, (pair, grp, hi) in enumerate(stream):
                    if pair not in ots:
                        ots[pair] = [psO.tile([128, 512], f32, tag="ot",
                                              name=f"ot{qj}{pair * 2 + j}")
                                     for j in range(2)]
                        n_done[pair] = [0, 0]
                    h = pair * 2 + hi
                    sc = psS.tile([128, 1024], f32, tag="score")
                    emit_scores(grp, h, sc)
                    pt = ptp.tile([128, 1024], f16, tag="pt")
                    emit_exp_mask(grp, pt, sc)
                    while (not defer_av
                           and (av_gate is None or av_gate[0])
                           and pend
                           and ((pend[0][0] == pair and len(pend) >= 2)
                                or len(pend) >= 3)):
                        flush_av()
                    pend.append((pair, grp, hi, h, pt))
                    if kw and idx >= len(stream) - 6:
                        pass    # keep the critical tail free of filler
                    else:
                        pull_filler(pulls)
                if defer_av:
                    # hand the AV+norm emission back to the caller as a
                    # filler generator (used for qj0, whose V data arrives
                    # long after its scores are done)
                    def flusher():
                        while pend:
                            flush_av()
                            yield
                    return flusher()
                if av_gate is not None:
                    # the gate orders our first AV after a predecessor
                    # q-block's norms; drain the rest of the filler if the
                    # item loop didn't get there
                    while not av_gate[0] and filler is not None:
                        try:
                            next(filler)
                        except StopIteration:
                            break
                while pend:
                    flush_av()
                if not defer_oproj:
                    emit_oproj_qblock(qj)

            # ---- output projection (partial over local e-dims) ----
            # generator yielding per half-tile; psA ring (transient, like
            # the projection pieces it interleaves with)
            def gen_oproj(qj, act_copy=False, gs=None):
                for g in (range(qj * 4, (qj + 1) * 4) if gs is None
                          else gs):
                    osr = ostp.tile([128, D], f16, tag="ost")
                    for jh in range(2):
                        # tail: psO is free once attention is done; using
                        # both rings doubles the po pipeline depth
                        pool = psO if (act_copy and (g + jh) % 2) else psA
                        po = pool.tile([128, 512], f32,
                                       tag=("ot" if pool is psO else "proj"),
                                       name=f"po{g}{jh}")
                        for ep in range(2):
                            nc.tensor.matmul(
                                po[:], OT_sb[:, ep, g * 128:(g + 1) * 128],
                                woT_sb[:, ep, jh * 512:(jh + 1) * 512],
                                start=(ep == 0), stop=(ep == 1))
                        if act_copy and jh == 0:
                            nc.scalar.copy(
                                osr[:, jh * 512:(jh + 1) * 512], po[:])
                        else:
                            nc.vector.tensor_copy(
                                osr[:, jh * 512:(jh + 1) * 512], po[:])
                        if act_copy:
                            # tail: stream each half out as soon as its
                            # copy lands; alternate issue queues so the
                            # copies and DMA issues don't serialize on
                            # one sequencer
                            eng = nc.scalar if jh == 0 else nc.sync
                            eng.dma_start(
                                out=out.ap()[g * 128:(g + 1) * 128,
                                             jh * 512:(jh + 1) * 512],
                                in_=osr[:, jh * 512:(jh + 1) * 512])
                        yield
                    if not act_copy:
                        nc.sync.dma_start(
                            out=out.ap()[g * 128:(g + 1) * 128, :],
                            in_=osr[:])

            def emit_oproj_qblock(qj, tail=False):
                if not tail:
                    drain(gen_oproj(qj))
                    return
                # tail: the ep0 matmuls only need the first head-pair's
                # norms (done mid-q-block), so emit them up front to fill
                # the PE while the last pair's norm chains run; ep1 +
                # copies pipeline behind on both psum rings.
                halves = [(g, jh) for g in range(qj * 4, (qj + 1) * 4)
                          for jh in range(2)]
                pos = {}
                osrs = {}

                def ep0(i):
                    g, jh = halves[i]
                    pool = psA if i % 2 == 0 else psO
                    po = pool.tile([128, 512], f32,
                                   tag=("proj" if pool is psA else "ot"),
                                   name=f"tpo{g}{jh}")
                    nc.tensor.matmul(
                        po[:], OT_sb[:, 0, g * 128:(g + 1) * 128],
                        woT_sb[:, 0, jh * 512:(jh + 1) * 512],
                        start=True, stop=False)
                    pos[i] = po

                for i in range(4):
                    ep0(i)
                for i, (g, jh) in enumerate(halves):
                    po = pos.pop(i)
                    nc.tensor.matmul(
                        po[:], OT_sb[:, 1, g * 128:(g + 1) * 128],
                        woT_sb[:, 1, jh * 512:(jh + 1) * 512],
                        start=False, stop=True)
                    if g not in osrs:
                        osrs[g] = ostp.tile([128, D], f16, tag="ost", name=f"tosr{g}")
                    osr = osrs[g]
                    if jh == 0:
                        nc.scalar.copy(osr[:, 0:512], po[:])
                    else:
                        nc.vector.tensor_copy(osr[:, 512:1024], po[:])
                    eng = nc.scalar if jh == 0 else nc.sync
                    eng.dma_start(
                        out=out.ap()[g * 128:(g + 1) * 128,
                                     jh * 512:(jh + 1) * 512],
                        in_=osr[:, jh * 512:(jh + 1) * 512])
                    if i + 4 < len(halves):
                        ep0(i + 4)

            # keep-warm: tiny write-only matmuls fill PE idle so following
            # matmuls run at full clock instead of re-ramping.
            def emit_keepwarm(n, tag0):
                for i in range(n):
                    kwt = psA.tile([64, 512], f32, tag="proj",
                                   name=f"kw{tag0}{i}")
                    nc.tensor.matmul(kwt[:, 0:256], woT_sb[:, 0, 0:64],
                                     woT_sb[:, 0, 0:256],
                                     start=True, stop=True)

            # ---- schedule ----
            # Startup: q/k e-tile 0 of chunk 0 projects first so qj0 pair A
            # attention starts as early as possible; the rest of chunk 0
            # fills its PE gaps. Then qj order [0B, 1, 2, 3] with the next
            # projection chunk and the previous q-block's oproj interleaved
            # into each attention loop as PE filler. The tail is qj3's last
            # norms + its oproj, streamed out per half-tile.
            emit_vz()
            # prologue: the startup is DMA-transfer-order bound. Strict
            # first-use order: wq+q0 (first matmuls), wk+k0 (first
            # scores), chunk-1 q/k (so the qj1 exps never wait), then the
            # v stream. qj0's AVs are deferred into attn(1)'s filler: the
            # exp stream runs continuously while V data is still in
            # flight, and the in-order PE queue never stalls on it.
            st0 = stagep.tile([128, 3, NDT, CH], f16, tag="stage")
            st1 = stagep.tile([128, 3, NDT, CH], f16, tag="stage")
            nc.scalar.dma_start(out=bq_sb[:], in_=bqd.ap()[:])
            nc.scalar.dma_start(out=aux_sb[:, 0:MW], in_=auxd.ap()[:, 0:MW])
            nc.sync.dma_start(out=w3[:, 0, 0:4], in_=w3_r[:, 0, 0:4])
            nc.sync.dma_start(out=st0[:, 0, 0:4], in_=qkv_r[:, 0, 0:4, 0:CH])
            nc.sync.dma_start(out=w3[:, 0, 4:NDT], in_=w3_r[:, 0, 4:NDT])
            nc.sync.dma_start(out=w3[:, 1], in_=w3_r[:, 1])
            nc.sync.dma_start(out=st0[:, 1, 0:4], in_=qkv_r[:, 1, 0:4, 0:CH])
            nc.sync.dma_start(out=st0[:, 0, 4:NDT],
                              in_=qkv_r[:, 0, 4:NDT, 0:CH])
            nc.sync.dma_start(out=st0[:, 1, 4:NDT],
                              in_=qkv_r[:, 1, 4:NDT, 0:CH])
            nc.sync.dma_start(out=st1[:, 0:2], in_=qkv_r[:, 0:2, :, CH:2 * CH])
            nc.sync.dma_start(out=w3[:, 2], in_=w3_r[:, 2])
            nc.sync.dma_start(out=st0[:, 2], in_=qkv_r[:, 2, :, 0:CH])
            nc.sync.dma_start(out=st1[:, 2], in_=qkv_r[:, 2, :, CH:2 * CH])
            nc.scalar.dma_start(out=aux_sb[:, MW:], in_=auxd.ap()[:, MW:])
            drain(gen_qk_ep("q", 0, 0, st0[:, 0], act=True))
            drain(gen_qk_ep("k", 0, 0, st0[:, 1], act=True))
            drain(gen_qk_ep("q", 0, 1, st0[:, 0], act=True))
            drain(gen_qk_ep("k", 0, 1, st0[:, 1], act=True))
            f0 = emit_attention(0, defer_oproj=True, defer_av=True,
                                act_oc=True)
            drain(gen_qk_ep("q", 1, 0, st1[:, 0]))
            drain(gen_qk_ep("k", 1, 0, st1[:, 1]))
            drain(gen_qk_ep("q", 1, 1, st1[:, 0]))
            drain(gen_qk_ep("k", 1, 1, st1[:, 1]))
            av_ok = [False]
            g1f = chain(gen_v(0, st0[:, 2]), f0,
                        flag_end(gen_v(1, st1[:, 2]), av_ok),
                        gen_chunk3(2))
            emit_attention(1, defer_oproj=True, filler=g1f, pulls=2,
                           av_gate=av_ok, kw=True)
            drain(g1f)
            g2 = chain(gen_oproj(0), gen_chunk3(3))
            emit_attention(2, defer_oproj=True, filler=g2, kw=True)
            drain(g2)
            emit_attention(3, defer_oproj=True, tail=True, kw=True,
                           filler=chain(gen_oproj(1), gen_oproj(2)))
            emit_oproj_qblock(3, tail=True)

    nc.compile()
    return nc


_CACHE = {}


def kernel(q, k, v, mask, wq, bq, wk, bk, wv, bv, wo, bo):
    import ml_dtypes
    from concourse.bass_utils import run_bass_kernel_spmd
    npf8 = ml_dtypes.float8_e4m3

    q = np.asarray(q, np.float32)
    k = np.asarray(k, np.float32)
    v = np.asarray(v, np.float32)
    wq = np.asarray(wq, np.float32)
    wk = np.asarray(wk, np.float32)
    wv = np.asarray(wv, np.float32)
    wo = np.asarray(wo, np.float32)
    bq = np.asarray(bq, np.float32)
    bv = np.asarray(bv, np.float32)
    bo = np.asarray(bo, np.float32)

    cls, rng, pr, pats = _classify_mask(mask)
    pat_widths = [p.shape[1] for p in pats]
    key = (tuple(tuple(r) for r in cls), tuple(tuple(r) for r in rng),
           tuple(tuple(r) for r in pr), tuple(pat_widths))
    if key not in _CACHE:
        _CACHE[key] = _build_program(cls, rng, pr, pat_widths)
    nc = _CACHE[key]

    if pats:
        masks_np = np.concatenate(pats, axis=1).astype(np.float16)
    else:
        masks_np = np.zeros((128, 1), np.float16)

    # per-batch transposed fp16 inputs, q/k/v packed into one tensor
    qkv8 = [np.ascontiguousarray(
        np.concatenate([q[b].T, k[b].T, v[b].T], axis=0)
        .astype(np.float16)) for b in range(B)]

    def pack_w3(hg):
        el = slice(hg * EL, (hg + 1) * EL)
        ws = []
        for w in (wq, wk, wv):
            wt = np.ascontiguousarray(w[el, :].T)         # [D, EL]
            ws.append(wt.reshape(NDT, 128, EL).transpose(1, 0, 2))
        return np.ascontiguousarray(
            np.stack(ws, axis=1).reshape(128, 3 * NDT * EL)
            .astype(np.float16))

    in_maps = []
    for c in range(NC):
        b, hg = c // 4, c % 4
        el = slice(hg * EL, (hg + 1) * EL)
        woTl = np.ascontiguousarray(wo[:, el].T.astype(np.float16))
        aux = np.concatenate(
            [masks_np,
             woTl.reshape(2, 128, D).transpose(1, 0, 2).reshape(128, 2 * D)],
            axis=1)
        m = {
            "qkv": qkv8[b],
            "w3": pack_w3(hg),
            "aux": np.ascontiguousarray(aux),
            "bq": np.ascontiguousarray(
                bq[el].reshape(2, 128).T),
        }
        in_maps.append(m)

    res = run_bass_kernel_spmd(nc, in_maps, list(range(NC)))
    accs = []
    for b in range(B):
        acc = res.results[b * 4]["out"].astype(np.float32)
        for hg in range(1, 4):
            acc = acc + res.results[b * 4 + hg]["out"]
        accs.append(acc)
    outf = np.stack(accs).reshape(B, S, D)
    # bo plus the folded V bias: softmax weights sum to 1 so the V bias
    # contributes wo @ bv to every output row
    outf = outf + (bo + wo @ bv)[None, None, :]
    return outf


# revision 65
# speedup vs baseline: 1.0029x; 1.0029x over previous
# Trainium2 Bass kernel for nn_MultiHeadAttention (B=2, S=2048, D=1024, H=16).
#
# Sharding: batch+head tensor-parallel over 8 cores. Core c handles batch
# c//4 and head-group c%4 (4 heads, 256 e-dims): column-sharded wq/wk/wv,
# row-sharded wo with the partial-output sum done on the host. Each core
# only reads its batch's q/k/v (12MB fp16 -> 6MB fp8 per core) and writes
# a [2048, 1024] fp16 partial.
#
# Projections run as fp8e4 DoubleRow matmuls (2 contraction tiles of 128
# per pass, 0.5 cyc/row): q/k/v stream fp8 from the host, wq/wk (and bq)
# pre-scaled by 16 so the fp8 weights stay clear of subnormals; the 16*16
# score scaling is folded into the softmax exp scale (0.125/256).
#
# Attention stays fp16 (DoubleRow dst must start at partition 0, which
# makes the Z ones-row trick impossible in DR mode):
#   QT/KT = (e 128, 2 e-tiles, 2048) computed via DR with weight tiles
#   stationary; V in natural (token, e) layout packed [V_h | ones] per
#   (kt, head) so AV yields row sums (Z) free in psum row 64.
#   ScoresT = (k, q) per head; exp PSUM->SBUF on ACT gives P^T directly.
#   Causal-boundary blocks multiply P^T with a resident 0/1 triangle tile
#   on DVE (fp16 2x mode).
# Bias handling: K bias dropped (softmax per-query shift invariance),
#   V bias folded into the host-side output bias (out += wo @ bv),
#   Q bias added on DVE during the PSUM->SBUF copy.
# Host sums the 4 partial outputs per batch in fp32 and adds bo + wo@bv.

import numpy as np

B, S, D, H = 2, 2048, 1024, 16
DK = D // H            # 64
NC = 8                 # cores
NH = 4                 # heads per core
EL = NH * DK           # 256 local e-dims
NCH = 4                # projection token-chunks per core
CH = S // NCH          # 512
NDT = D // 128         # 8 contraction tiles
NDP = NDT // 2         # 4 DoubleRow contraction pairs
NKT = S // 128         # 16 k-tiles
NQB = S // 512         # 4 q-blocks

SKIP, PLAIN = -1, -2   # block classes (>=0 means partial-pattern index)


def _classify_mask(mask):
    """Per (kt, qj) block classification of the (S_q, S_k) mask.

    Returns cls[kt][qj] (SKIP / PLAIN / pattern idx), rng[kt][qj] live col
    range, pr[kt][qj] partial col range, and the deduped 0/1 patterns
    (list of [128, w] float16 arrays) for the partial ranges."""
    m = np.asarray(mask).reshape(S, S)              # [q, k]; 0 = masked
    liveT = (m != 0).T                              # [k, q]
    cls = [[PLAIN] * NQB for _ in range(NKT)]
    rng = [[(0, 512)] * NQB for _ in range(NKT)]
    pr = [[(0, 0)] * NQB for _ in range(NKT)]
    uniq = {}
    pats = []
    for kt in range(NKT):
        for qj in range(NQB):
            blk = liveT[kt * 128:(kt + 1) * 128, qj * 512:(qj + 1) * 512]
            if blk.all():
                cls[kt][qj] = PLAIN
            elif not blk.any():
                cls[kt][qj] = SKIP
            else:
                live_col = blk.any(axis=0)
                nz = np.nonzero(live_col)[0]
                c0, c1 = int(nz[0]), int(nz[-1]) + 1
                rng[kt][qj] = (c0, c1)
                part_col = live_col & ~blk.all(axis=0)
                pz = np.nonzero(part_col)[0]
                p0, p1 = int(pz[0]), int(pz[-1]) + 1
                pr[kt][qj] = (p0, p1)
                pat = blk[:, p0:p1].astype(np.float16)
                key = (p1 - p0, pat.tobytes())
                if key not in uniq:
                    uniq[key] = len(pats)
                    pats.append(np.ascontiguousarray(pat))
                cls[kt][qj] = uniq[key]
    return cls, rng, pr, pats


def _build_program(cls, rng, pr, pat_widths):
    import concourse.bacc as bacc
    import concourse.mybir as mybir
    from concourse.tile import TileContext

    f32 = mybir.dt.float32
    f16 = mybir.dt.float16
    f8 = mybir.dt.float8e4
    DR = mybir.MatmulPerfMode.DoubleRow
    Exp = mybir.ActivationFunctionType.Exp
    mult = mybir.AluOpType.mult
    ESC = 0.125            # exp scale: 1/sqrt(DK)

    # pattern offsets inside the resident mask tile
    moff = []
    o = 0
    for w in pat_widths:
        moff.append(o)
        o += w
    MW = max(o, 1)

    nc = bacc.Bacc("TRN2", target_bir_lowering=False, debug=False,
                   num_devices=NC)

    # q/k/v packed in one tensor so a projection chunk is a single DMA
    # (the per-DMA issue pipeline -- seq config + HWDGE + completion
    # semaphore -- costs ~1.5us, which dominated the startup)
    qkv = nc.dram_tensor("qkv", [3 * D, S], f16, kind="ExternalInput")
    w3d = nc.dram_tensor("w3", [128, 3 * NDT * EL], f16,
                         kind="ExternalInput")
    bqd = nc.dram_tensor("bq", [128, 2], f32, kind="ExternalInput")
    auxd = nc.dram_tensor("aux", [128, MW + 2 * D], f16,
                          kind="ExternalInput")
    out = nc.dram_tensor("out", [S, D], f16, kind="ExternalOutput")

    # transposed-input view: [p, j, t, c] with t the 128-row d-block
    qkv_r = qkv.ap().rearrange("(j t p) c -> p j t c", j=3, p=128)
    w3_r = w3d.ap().rearrange("p (j t e) -> p j t e", j=3, t=NDT)

    with TileContext(nc) as tc:
        with (
            tc.tile_pool(name="const", bufs=1) as constp,
            tc.tile_pool(name="per", bufs=1) as perp,
            tc.tile_pool(name="stage", bufs=2) as stagep,
            tc.tile_pool(name="pt", bufs=18) as ptp,
            tc.tile_pool(name="zz", bufs=4) as zzp,
            tc.tile_pool(name="zb", bufs=6) as zbp,
            tc.tile_pool(name="ost", bufs=4) as ostp,
            tc.tile_pool(name="psA", bufs=2, space="PSUM") as psA,
            tc.tile_pool(name="psS", bufs=2, space="PSUM") as psS,
            tc.tile_pool(name="psO", bufs=2, space="PSUM") as psO,
        ):
            # ---- constants ----
            w3 = constp.tile([128, 3, NDT, EL], f16, tag="w3")
            bq_sb = constp.tile([128, 2], f32, tag="bq")
            aux_sb = constp.tile([128, MW + 2 * D], f16, tag="aux")
            msk = aux_sb[:, 0:MW]
            woT_sb = aux_sb[:, MW:].rearrange("p (i e) -> p i e", i=2)

            # ---- persistent activations ----
            # QT/KT/OT: [p, e-tile, token]; head h -> (rows (h%2)*64,
            # e-tile h//2)
            QT_sb = perp.tile([128, 2, S], f16, tag="QT")
            KT_sb = perp.tile([128, 2, S], f16, tag="KT")
            OT_sb = perp.tile([128, 2, S], f16, tag="OT")
            # V natural (token, e) packed per (kt, head) as
            # [ones-col | 63 zero cols | V_h(64)] (128 cols): Z (the exp
            # row sum) lands in psum row 0 where the DVE reciprocal can
            # read it without a staging copy, and the AV rows sit at
            # partitions 64:128 (64-partition engine APs must start at 0
            # or 64 on hardware).
            V_big = perp.tile([128, NKT * NH * 128], f16, tag="Vb")
            V3 = V_big[:].rearrange("p (t x) -> p t x", x=128)

            def emit_vz():
                nc.vector.memset(V3[:, :, 0:64], 0.0)
                nc.vector.memset(V3[:, :, 0:1], 1.0)

            # ---- projections (fp8 DoubleRow, 2 d-tiles per pass) ----
            # generators yield after each PE quantum (~0.2-0.4us) so the
            # attention loop can drain them as PE filler between groups
            JIDX = {"q": 0, "k": 1, "v": 2}

            def stage_chunk(c):
                # whole q/k/v chunk in ONE DMA from the packed qkv tensor
                lo = c * CH
                st3 = stagep.tile([128, 3, NDT, CH], f16, tag="stage")
                nc.sync.dma_start(out=st3[:], in_=qkv_r[:, :, :, lo:lo + CH])
                return st3

            def gen_qk_ep(name, c, ep, st, act=False):
                # transposed layout [e, token]: out [128 e, 512] per
                # e-tile, weight tiles stationary. act=True routes the
                # PSUM->SBUF copy to the ACT engine (used in the startup
                # where DVE is the bottleneck and ACT is idle).
                w = w3[:, JIDX[name]]
                lo = c * CH
                ps = psA.tile([128, 512], f32, tag="proj",
                              name=f"{name}p{c}{ep}")
                for t in range(NDT):
                    nc.tensor.matmul(
                        ps[:], w[:, t, ep * 128:(ep + 1) * 128],
                        st[:, t, :],
                        start=(t == 0), stop=(t == NDT - 1))
                    if t == 3:
                        yield
                if name == "q":
                    if act:
                        nc.scalar.add(QT_sb[:, ep, lo:lo + CH], ps[:],
                                      bq_sb[:, ep:ep + 1])
                    else:
                        nc.vector.tensor_scalar_add(
                            QT_sb[:, ep, lo:lo + CH], ps[:],
                            bq_sb[:, ep:ep + 1])
                elif act:
                    nc.scalar.copy(KT_sb[:, ep, lo:lo + CH], ps[:])
                else:
                    nc.vector.tensor_copy(KT_sb[:, ep, lo:lo + CH], ps[:])
                yield

            def gen_v(c, st, act=False):
                # natural layout: out [64 tok, 256 e] per subtile; a psum
                # [128, 512] holds one kt (128 tokens) per col-half
                w = w3[:, 2]
                for half in range(2):          # 2 kt per psum tile
                    ps = psA.tile([128, 512], f32, tag="proj",
                                  name=f"vp{c}{half}")
                    for kk in range(2):        # kt within this psum tile
                        tt = half * 2 + kk
                        sub = ps[:, kk * 256:(kk + 1) * 256]
                        for t in range(NDT):
                            nc.tensor.matmul(
                                sub, st[:, t, tt * 128:(tt + 1) * 128],
                                w[:, t, :],
                                start=(t == 0), stop=(t == NDT - 1))
                        yield
                    for kk in range(2):
                        kt = c * 4 + half * 2 + kk
                        dst = V3[:, kt * NH:(kt + 1) * NH, 64:128]
                        src = ps[:, kk * 256:(kk + 1) * 256].rearrange(
                            "p (h e) -> p h e", h=NH)
                        if act:
                            nc.scalar.copy(dst, src)
                        else:
                            nc.vector.tensor_copy(dst, src)

            def gen_chunk3(c, staged=None, act=False):
                # e-tile 0 of q AND k first: the next q-block's pair-A
                # scores need exactly those
                st3 = stage_chunk(c) if staged is None else staged
                yield from gen_qk_ep("q", c, 0, st3[:, 0], act=act)
                yield from gen_qk_ep("k", c, 0, st3[:, 1], act=act)
                yield from gen_qk_ep("q", c, 1, st3[:, 0], act=act)
                yield from gen_qk_ep("k", c, 1, st3[:, 1], act=act)
                yield from gen_v(c, st3[:, 2], act=act)

            def chain(*gens):
                for g in gens:
                    yield from g

            def flag_end(gen, flag):
                yield from gen
                flag[0] = True

            def drain(g):
                for _ in g:
                    pass

            # ---- attention ----
            # Per q-block, heads processed as two interleaved pairs so PE
            # always has the other head's matmuls while ACT runs exp. The
            # AV matmul lags scores by one group so it never waits on exp,
            # and a caller-supplied filler (projection / oproj pieces) is
            # drained between groups to keep the PE queue dense.
            def emit_attention(qj, pairs=(0, 1), defer_oproj=False,
                               tail=False, kw=False, filler=None, pulls=1,
                               act_oc=False, defer_av=False, av_gate=None):
                qlo = qj * 512
                acts = [kt for kt in range(NKT) if cls[kt][qj] != SKIP]
                if not acts:
                    return
                groups = [acts[gi:gi + 2] for gi in range(0, len(acts), 2)]

                def pull_filler(n):
                    for _ in range(n):
                        if filler is not None:
                            try:
                                next(filler)
                                continue
                            except StopIteration:
                                pass
                        if kw:
                            emit_keepwarm(2, 700 + qj * 100
                                          + 3 * pull_filler.kwn)
                            pull_filler.kwn += 1
                        break
                pull_filler.kwn = 0

                def emit_scores(grp, h, sc):
                    ep, hp = h // 2, (h % 2) * 64
                    for i, kt in enumerate(grp):
                        c0, c1 = rng[kt][qj]
                        klo = kt * 128
                        nc.tensor.matmul(
                            sc[:, i * 512 + c0:i * 512 + c1],
                            KT_sb[hp:hp + 64, ep, klo:klo + 128],
                            QT_sb[hp:hp + 64, ep, qlo + c0:qlo + c1],
                            start=True, stop=True)

                def emit_exp_mask(grp, pt, sc):
                    spans = [(i * 512 + rng[kt][qj][0],
                              i * 512 + rng[kt][qj][1])
                             for i, kt in enumerate(grp)]
                    lo, hi2 = spans[0][0], spans[-1][1]
                    dead = (hi2 - lo) - sum(b - a for a, b in spans)
                    exp_spans = spans if dead > 0 else [(lo, hi2)]
                    for a, bnd in exp_spans:
                        nc.scalar.activation(pt[:, a:bnd], sc[:, a:bnd],
                                             Exp, scale=ESC)
                    for i, kt in enumerate(grp):
                        cl = cls[kt][qj]
                        if cl >= 0:
                            pp0, pp1 = pr[kt][qj]
                            sl = slice(i * 512 + pp0, i * 512 + pp1)
                            nc.vector.tensor_tensor(
                                pt[:, sl], pt[:, sl],
                                msk[:, moff[cl]:moff[cl] + pp1 - pp0],
                                op=mult)

                def emit_av(grp, hi, h, pt, ots, n_done):
                    for i, kt in enumerate(grp):
                        c0, c1 = rng[kt][qj]
                        base = (kt * NH + h) * 128
                        vap = V_big[:, base:base + 128]
                        n_done[hi] += 1
                        nc.tensor.matmul(
                            ots[hi][:, c0:c1], vap,
                            pt[:, i * 512 + c0:i * 512 + c1],
                            start=(n_done[hi] == 1),
                            stop=(n_done[hi] == len(acts)))

                def emit_norm(pair, hi, h, ots):
                    # normalize: psum row 0 of ot = Z (the ones column),
                    # rows 64:128 = AV. The reciprocal reads Z straight
                    # from PSUM partition 0 (no staging copy); the AV rows
                    # move to SBUF base 0 so the normalize tensor_tensor
                    # sees equal base partitions on its two SBUF inputs.
                    ep, hp = h // 2, (h % 2) * 64
                    ot = ots[hi]
                    on_act = act_oc or (tail and pair == pairs[-1])
                    z = zzp.tile([1, 512], f32, tag="z")
                    if on_act:
                        nc.scalar.copy(z[:], ot[0:1, :])
                    else:
                        nc.vector.tensor_copy(z[:], ot[0:1, :])
                    rz = zzp.tile([1, 512], f32, tag="z")
                    nc.vector.reciprocal_approx_fast(rz[:], z[:])
                    oc = zbp.tile([64, 512], f32, tag="oc")
                    if on_act:
                        # ACT is idle in the startup and the tail
                        nc.scalar.copy(oc[:], ot[64:128, :])
                    else:
                        nc.vector.tensor_copy(oc[:], ot[64:128, :])
                    rb = zbp.tile([64, 512], f32, tag="zb")
                    nc.gpsimd.partition_broadcast(rb[:], rz[:],
                                                  channels=64)
                    if tail and pair == pairs[-1]:
                        # keep the PE clock warm through the norm chain:
                        # these matmuls depend on oc so they execute
                        # spread across the DVE/Pool chain, not up front
                        for kwi in range(2):
                            kwt = psA.tile([64, 512], f32, tag="proj",
                                           name=f"kwn{qj}{h}{kwi}")
                            nc.tensor.matmul(
                                kwt[:, 0:64], oc[:, 0:64],
                                oc[:, 0:64], start=True, stop=True)
                    nc.vector.tensor_tensor(
                        OT_sb[hp:hp + 64, ep, qlo:qlo + 512],
                        oc[:, :], rb[:], op=mult)

                # Flat item stream over both pairs. The AV for an item
                # lags its exp so the PE never waits on ACT; across the
                # pair boundary the lag stretches to 2 so pair A's last
                # AVs + norms are all emitted BEFORE pair B's first AV
                # touches the psO ring slot that pair A's norms still
                # read (emission order is dependency order).
                stream = [(pair, grp, hi) for pair in pairs
                          for grp in groups for hi in range(2)]

                ots = {}
                n_done = {}
                n_av = {p: 0 for p in pairs}
                pend = []   # (pair, grp, hi, h, pt)

                def flush_av():
                    pair, grp, hi, h, pt = pend.pop(0)
                    emit_av(grp, hi, h, pt, ots[pair], n_done[pair])
                    n_av[pair] += len(grp)
                    if n_av[pair] == 2 * len(acts):
                        for hj in range(2):
                            emit_norm(pair, hj, pair * 2 + hj, ots[pair])

                for idx, (pair, grp, hi) in enumerate(stream):
                    if pair not in ots:
                        ots[pair] = [psO.tile([128, 512], f32, tag="ot",
                                              name=f"ot{qj}{pair * 2 + j}")
                                     for j in range(2)]
                        n_done[pair] = [0, 0]
                    h = pair * 2 + hi
                    sc = psS.tile([128, 1024], f32, tag="score")
                    emit_scores(grp, h, sc)
                    pt = ptp.tile([128, 1024], f16, tag="pt")
                    emit_exp_mask(grp, pt, sc)
                    while (not defer_av
                           and (av_gate is None or av_gate[0])
                           and pend
                           and ((pend[0][0] == pair and len(pend) >= 2)
                                or len(pend) >= 3)):
                        flush_av()
                    pend.append((pair, grp, hi, h, pt))
                    if kw and idx >= len(stream) - 6:
                        pass    # keep the critical tail free of filler
                    else:
                        pull_filler(pulls)
                if defer_av:
                    # hand the AV+norm emission back to the caller as a
                    # filler generator (used for qj0, whose V data arrives
                    # long after its scores are done)
                    def flusher():
                        while pend:
                            flush_av()
                            yield
                    return flusher()
                if av_gate is not None:
                    # the gate orders our first AV after a predecessor
                    # q-block's norms; drain the rest of the filler if the
                    # item loop didn't get there
                    while not av_gate[0] and filler is not None:
                        try:
                            next(filler)
                        except StopIteration:
                            break
                while pend:
                    flush_av()
                if not defer_oproj:
                    emit_oproj_qblock(qj)

            # ---- output projection (partial over local e-dims) ----
            # generator yielding per half-tile; psA ring (transient, like
            # the projection pieces it interleaves with)
            def gen_oproj(qj, act_copy=False, gs=None):
                for g in (range(qj * 4, (qj + 1) * 4) if gs is None
                          else gs):
                    osr = ostp.tile([128, D], f16, tag="ost")
                    for jh in range(2):
                        # tail: psO is free once attention is done; using
                        # both rings doubles the po pipeline depth
                        pool = psO if (act_copy and (g + jh) % 2) else psA
                        po = pool.tile([128, 512], f32,
                                       tag=("ot" if pool is psO else "proj"),
                                       name=f"po{g}{jh}")
                        for ep in range(2):
                            nc.tensor.matmul(
                                po[:], OT_sb[:, ep, g * 128:(g + 1) * 128],
                                woT_sb[:, ep, jh * 512:(jh + 1) * 512],
                                start=(ep == 0), stop=(ep == 1))
                        if act_copy and jh == 0:
                            nc.scalar.copy(
                                osr[:, jh * 512:(jh + 1) * 512], po[:])
                        else:
                            nc.vector.tensor_copy(
                                osr[:, jh * 512:(jh + 1) * 512], po[:])
                        if act_copy:
                            # tail: stream each half out as soon as its
                            # copy lands; alternate issue queues so the
                            # copies and DMA issues don't serialize on
                            # one sequencer
                            eng = nc.scalar if jh == 0 else nc.sync
                            eng.dma_start(
                                out=out.ap()[g * 128:(g + 1) * 128,
                                             jh * 512:(jh + 1) * 512],
                                in_=osr[:, jh * 512:(jh + 1) * 512])
                        yield
                    if not act_copy:
                        nc.sync.dma_start(
                            out=out.ap()[g * 128:(g + 1) * 128, :],
                            in_=osr[:])

            def emit_oproj_qblock(qj, tail=False):
                if not tail:
                    drain(gen_oproj(qj))
                    return
                # tail: the ep0 matmuls only need the first head-pair's
                # norms (done mid-q-block), so emit them up front to fill
                # the PE while the last pair's norm chains run; ep1 +
                # copies pipeline behind on both psum rings.
                halves = [(g, jh) for g in range(qj * 4, (qj + 1) * 4)
                          for jh in range(2)]
                pos = {}
                osrs = {}

                def ep0(i):
                    g, jh = halves[i]
                    pool = psA if i % 2 == 0 else psO
                    po = pool.tile([128, 512], f32,
                                   tag=("proj" if pool is psA else "ot"),
                                   name=f"tpo{g}{jh}")
                    nc.tensor.matmul(
                        po[:], OT_sb[:, 0, g * 128:(g + 1) * 128],
                        woT_sb[:, 0, jh * 512:(jh + 1) * 512],
                        start=True, stop=False)
                    pos[i] = po

                for i in range(4):
                    ep0(i)
                for i, (g, jh) in enumerate(halves):
                    po = pos.pop(i)
                    nc.tensor.matmul(
                        po[:], OT_sb[:, 1, g * 128:(g + 1) * 128],
                        woT_sb[:, 1, jh * 512:(jh + 1) * 512],
                        start=False, stop=True)
                    if g not in osrs:
                        osrs[g] = ostp.tile([128, D], f16, tag="ost", name=f"tosr{g}")
                    osr = osrs[g]
                    if jh == 0:
                        nc.scalar.copy(osr[:, 0:512], po[:])
                    else:
                        nc.vector.tensor_copy(osr[:, 512:1024], po[:])
                    eng = nc.scalar if jh == 0 else nc.sync
                    eng.dma_start(
                        out=out.ap()[g * 128:(g + 1) * 128,
                                     jh * 512:(jh + 1) * 512],
                        in_=osr[:, jh * 512:(jh + 1) * 512])
                    if i + 4 < len(halves):
                        ep0(i + 4)

            # keep-warm: tiny write-only matmuls fill PE idle so following
            # matmuls run at full clock instead of re-ramping.
            def emit_keepwarm(n, tag0):
                for i in range(n):
                    kwt = psA.tile([64, 512], f32, tag="proj",
                                   name=f"kw{tag0}{i}")
                    nc.tensor.matmul(kwt[:, 0:256], woT_sb[:, 0, 0:64],
                                     woT_sb[:, 0, 0:256],
                                     start=True, stop=True)

            # ---- schedule ----
            # Startup: q/k e-tile 0 of chunk 0 projects first so qj0 pair A
            # attention starts as early as possible; the rest of chunk 0
            # fills its PE gaps. Then qj order [0B, 1, 2, 3] with the next
            # projection chunk and the previous q-block's oproj interleaved
            # into each attention loop as PE filler. The tail is qj3's last
            # norms + its oproj, streamed out per half-tile.
            emit_vz()
            # prologue: the startup is DMA-transfer-order bound. Strict
            # first-use order: wq+q0 (first matmuls), wk+k0 (first
            # scores), chunk-1 q/k (so the qj1 exps never wait), then the
            # v stream. qj0's AVs are deferred into attn(1)'s filler: the
            # exp stream runs continuously while V data is still in
            # flight, and the in-order PE queue never stalls on it.
            st0 = stagep.tile([128, 3, NDT, CH], f16, tag="stage")
            st1 = stagep.tile([128, 3, NDT, CH], f16, tag="stage")
            nc.scalar.dma_start(out=bq_sb[:], in_=bqd.ap()[:])
            nc.scalar.dma_start(out=aux_sb[:, 0:MW], in_=auxd.ap()[:, 0:MW])
            nc.sync.dma_start(out=w3[:, 0], in_=w3_r[:, 0])
            nc.sync.dma_start(out=st0[:, 0, 0:4], in_=qkv_r[:, 0, 0:4, 0:CH])
            nc.sync.dma_start(out=w3[:, 1], in_=w3_r[:, 1])
            nc.sync.dma_start(out=st0[:, 1, 0:4], in_=qkv_r[:, 1, 0:4, 0:CH])
            nc.sync.dma_start(out=st0[:, 0, 4:NDT],
                              in_=qkv_r[:, 0, 4:NDT, 0:CH])
            nc.sync.dma_start(out=st0[:, 1, 4:NDT],
                              in_=qkv_r[:, 1, 4:NDT, 0:CH])
            nc.sync.dma_start(out=st1[:, 0:2], in_=qkv_r[:, 0:2, :, CH:2 * CH])
            nc.sync.dma_start(out=w3[:, 2], in_=w3_r[:, 2])
            nc.sync.dma_start(out=st0[:, 2], in_=qkv_r[:, 2, :, 0:CH])
            nc.sync.dma_start(out=st1[:, 2], in_=qkv_r[:, 2, :, CH:2 * CH])
            nc.scalar.dma_start(out=aux_sb[:, MW:], in_=auxd.ap()[:, MW:])
            drain(gen_qk_ep("q", 0, 0, st0[:, 0], act=True))
            drain(gen_qk_ep("k", 0, 0, st0[:, 1], act=True))
            drain(gen_qk_ep("q", 0, 1, st0[:, 0], act=True))
            drain(gen_qk_ep("k", 0, 1, st0[:, 1], act=True))
            f0 = emit_attention(0, defer_oproj=True, defer_av=True,
                                act_oc=True)
            drain(gen_qk_ep("q", 1, 0, st1[:, 0]))
            drain(gen_qk_ep("k", 1, 0, st1[:, 1]))
            drain(gen_qk_ep("q", 1, 1, st1[:, 0]))
            drain(gen_qk_ep("k", 1, 1, st1[:, 1]))
            av_ok = [False]
            g1f = chain(gen_v(0, st0[:, 2]), f0,
                        flag_end(gen_v(1, st1[:, 2]), av_ok),
                        gen_chunk3(2))
            emit_attention(1, defer_oproj=True, filler=g1f, pulls=2,
                           av_gate=av_ok, kw=True)
            drain(g1f)
            g2 = chain(gen_oproj(0), gen_chunk3(3))
            emit_attention(2, defer_oproj=True, filler=g2, kw=True)
            drain(g2)
            emit_attention(3, defer_oproj=True, tail=True, kw=True,
                           filler=chain(gen_oproj(1), gen_oproj(2)))
            emit_oproj_qblock(3, tail=True)

    nc.compile()
    return nc


_CACHE = {}


def kernel(q, k, v, mask, wq, bq, wk, bk, wv, bv, wo, bo):
    import ml_dtypes
    from concourse.bass_utils import run_bass_kernel_spmd
    npf8 = ml_dtypes.float8_e4m3

    q = np.asarray(q, np.float32)
    k = np.asarray(k, np.float32)
    v = np.asarray(v, np.float32)
    wq = np.asarray(wq, np.float32)
    wk = np.asarray(wk, np.float32)
    wv = np.asarray(wv, np.float32)
    wo = np.asarray(wo, np.float32)
    bq = np.asarray(bq, np.float32)
    bv = np.asarray(bv, np.float32)
    bo = np.asarray(bo, np.float32)

    cls, rng, pr, pats = _classify_mask(mask)
    pat_widths = [p.shape[1] for p in pats]
    key = (tuple(tuple(r) for r in cls), tuple(tuple(r) for r in rng),
           tuple(tuple(r) for r in pr), tuple(pat_widths))
    if key not in _CACHE:
        _CACHE[key] = _build_program(cls, rng, pr, pat_widths)
    nc = _CACHE[key]

    if pats:
        masks_np = np.concatenate(pats, axis=1).astype(np.float16)
    else:
        masks_np = np.zeros((128, 1), np.float16)

    # per-batch transposed fp16 inputs, q/k/v packed into one tensor
    qkv8 = [np.ascontiguousarray(
        np.concatenate([q[b].T, k[b].T, v[b].T], axis=0)
        .astype(np.float16)) for b in range(B)]

    def pack_w3(hg):
        el = slice(hg * EL, (hg + 1) * EL)
        ws = []
        for w in (wq, wk, wv):
            wt = np.ascontiguousarray(w[el, :].T)         # [D, EL]
            ws.append(wt.reshape(NDT, 128, EL).transpose(1, 0, 2))
        return np.ascontiguousarray(
            np.stack(ws, axis=1).reshape(128, 3 * NDT * EL)
            .astype(np.float16))

    in_maps = []
    for c in range(NC):
        b, hg = c // 4, c % 4
        el = slice(hg * EL, (hg + 1) * EL)
        woTl = np.ascontiguousarray(wo[:, el].T.astype(np.float16))
        aux = np.concatenate(
            [masks_np,
             woTl.reshape(2, 128, D).transpose(1, 0, 2).reshape(128, 2 * D)],
            axis=1)
        m = {
            "qkv": qkv8[b],
            "w3": pack_w3(hg),
            "aux": np.ascontiguousarray(aux),
            "bq": np.ascontiguousarray(
                bq[el].reshape(2, 128).T),
        }
        in_maps.append(m)

    res = run_bass_kernel_spmd(nc, in_maps, list(range(NC)))
    accs = []
    for b in range(B):
        acc = res.results[b * 4]["out"].astype(np.float32)
        for hg in range(1, 4):
            acc = acc + res.results[b * 4 + hg]["out"]
        accs.append(acc)
    outf = np.stack(accs).reshape(B, S, D)
    # bo plus the folded V bias: softmax weights sum to 1 so the V bias
    # contributes wo @ bv to every output row
    outf = outf + (bo + wo @ bv)[None, None, :]
    return outf
